# revision 8
# baseline (speedup 1.0000x reference)
"""LoRA linear layer kernel for 8x Trainium2 NeuronCores.

Math: y = x @ W.T + b + ((x @ lora_b) @ lora_a) * (alpha/rank)
    = x @ (W.T + s*lora_b@lora_a) + b          with s = alpha/rank
so the LoRA factors fold into the weight on the host (exact identity).

Sharding: data-parallel over batch (8 batches -> 8 cores). Each core:
  yT_c[o, t] = sum_d WT[d, o] * xT_c[d, t] + b[o]
with xT_c = x[c].T so the contraction dim d sits on SBUF partitions for
both operands (PE matmul computes lhsT.T @ rhs with K on partitions).

Device layout per core (all hardcoded for D=4096, T=2048):
  xt   [4096, 2048] bf16  -- x[c].T, fully SBUF-resident (16 MiB)
  wb   [32, 128, 32, 128] bf16 -- WT blocked [m, p, k, c]:
        wb[m, p, k, c] = WT[k*128+p, m*128+c]; per-m block is a single
        contiguous 1 MiB DMA with 8 KiB per-partition lines.
  bias [128, 32] f32      -- bias[p, m] = b[m*128+p]
  yt   [4096, 2048] f32   -- output transposed; host untransposes.

Loop: for m (32 output-feature tiles): DMA W block; for n (4 token
tiles of 512): accumulate 32 k-matmuls into one PSUM bank, bias-add on
ScalarE, DMA out. PSUM pool bufs=8 double-buffers across m.
"""

import sys

for _p in ("/opt/trn_rl_repo",):
    if _p not in sys.path:
        sys.path.insert(0, _p)

import ml_dtypes
import numpy as np

import concourse.bass as bass
import concourse.mybir as mybir
import concourse.tile as tile
from concourse.bass import ts
from concourse.bass_utils import run_bass_kernel_spmd


def _split_multiwait_json(raw: bytes) -> bytes:
    """This walrus build allows at most ONE sem-wait per instruction
    (codegen setupSyncWait: "Too many sync wait commands"). Tile emits
    instructions with 2-3 waits. Split: hoist all but the last wait onto
    fresh single-wait NoOps on the same engine, inserted immediately
    before the instruction (same-engine program order preserved)."""
    import json as _json

    m = _json.loads(raw)
    next_id = [0]
    for f in m.get("functions", []):
        for b in f.get("blocks", []):
            for i in b.get("instructions", []):
                nm = i.get("name", "")
                if nm.startswith("I-"):
                    try:
                        next_id[0] = max(next_id[0], int(nm[2:]) + 1)
                    except ValueError:
                        pass
    for f in m.get("functions", []):
        for b in f.get("blocks", []):
            insts = b.get("instructions", [])
            out = []
            changed = False
            for i in insts:
                si = i.get("sync_info")
                ow = (si or {}).get("on_wait") or []
                if len(ow) > 1:
                    changed = True
                    for w in ow[:-1]:
                        out.append({
                            "debug": i.get("debug", 0),
                            "engine": i["engine"],
                            "ins": [],
                            "name": f"I-{next_id[0]}",
                            "opcode": "NoOp",
                            "outs": [],
                            "sync_info": {"on_update": [], "on_wait": [w]},
                        })
                        next_id[0] += 1
                    si["on_wait"] = [ow[-1]]
                out.append(i)
            if changed:
                b["instructions"] = out
    return _json.dumps(m).encode()


_orig_to_json_bytes = bass.Bass.to_json_bytes


def _to_json_bytes_patched(self):
    return _split_multiwait_json(_orig_to_json_bytes(self))


if not getattr(bass.Bass, "_multiwait_patched", False):
    bass.Bass.to_json_bytes = _to_json_bytes_patched
    bass.Bass._multiwait_patched = True


def _patched_drain_and_barrier(self, tick_clock, wait_clock):
    # This walrus build rejects >1 sem-wait on a CTRL_NO (Drain/Nop)
    # instruction; Tile's kernel-tail drain collects the whole global
    # clock onto one Drain. Attach the waits to a probe NOP instead and
    # redistribute so every CTRL op carries at most one wait.
    nc = self.nc
    probe = nc.sync.nop(nofuse=True)
    wait_clock.add_sem_waits(
        probe.ins, tile.ScopedClock({None: tick_clock.global_clock})
    )
    si = probe.ins.sync_info
    waits = list(si.on_wait) if si is not None and si.on_wait else []
    if len(waits) > 1:
        si.on_wait = waits[:1]
        for w in waits[1:]:
            extra = nc.sync.nop(nofuse=True)
            esi = extra.ins.sync_info
            if esi is None:
                extra.ins.sync_info = mybir.SyncInfo(on_wait=[w], on_update=[])
            else:
                esi.on_wait = [w]
    nc.sync.drain()

    nc.all_engine_barrier()
    assert self.sems is not None
    popped = nc._tile_sem_poison_stack.pop()
    assert popped is self._sem_poison
    nc.clear_and_free_semaphores(list(self.sems.allocated().values()))
    nc.all_engine_barrier()


tile.TileContext._drain_and_barrier = _patched_drain_and_barrier

N_CORES = 8
D = 4096
T = 2048          # tokens per core (one batch element)
P = 128
KT = D // P       # 32 contraction tiles
MT = D // P       # 32 output-feature tiles
NT = T // 512     # 4 token tiles of 512
SCALE = 16.0 / 8.0

BF16 = mybir.dt.bfloat16
F32 = mybir.dt.float32


def build_nc(w_dtype=BF16, x_dtype=BF16, k_outer=True):
    nc = bass.Bass()
    xt = nc.dram_tensor("xt", [D, T], x_dtype, kind="ExternalInput")
    wb = nc.dram_tensor("wb", [MT, P, KT, P], w_dtype, kind="ExternalInput")
    bias = nc.dram_tensor("bias", [P, MT], F32, kind="ExternalInput")
    yt = nc.dram_tensor("yt", [D, T], F32, kind="ExternalOutput")

    with tile.TileContext(nc) as tc:
        with (
            tc.tile_pool(name="xpool", bufs=1) as xpool,
            tc.tile_pool(name="wpool", bufs=3) as wpool,
            tc.tile_pool(name="bpool", bufs=1) as bpool,
            tc.tile_pool(name="opool", bufs=8) as opool,
            tc.tile_pool(name="psum", bufs=(2 if k_outer else 8), space="PSUM") as psum,
        ):
            bt = bpool.tile([P, MT], F32)
            nc.sync.dma_start(bt[:], bias[:])

            # x fully resident: one tile per k-slice so matmuls only wait
            # on the slice they read.
            xtiles = []
            for k in range(KT):
                xk = xpool.tile([P, T], x_dtype, tag=f"x{k}")
                nc.sync.dma_start(xk[:], xt[ts(k, P), :])
                xtiles.append(xk)

            for m in range(MT):
                w = wpool.tile([P, KT, P], w_dtype)
                nc.sync.dma_start(w[:], wb[m])
                if k_outer:
                    # weight tile (m,k) feeds 4 consecutive matmuls
                    pss = [
                        psum.tile([P, 512], F32, name=f"ps{n}", tag=f"ps{n}")
                        for n in range(NT)
                    ]
                    for k in range(KT):
                        for n in range(NT):
                            nc.tensor.matmul(
                                pss[n][:],
                                lhsT=w[:, k, :],
                                rhs=xtiles[k][:, ts(n, 512)],
                                start=(k == 0),
                                stop=(k == KT - 1),
                            )
                    for n in range(NT):
                        ot = opool.tile([P, 512], F32)
                        nc.scalar.activation(
                            ot[:], pss[n][:],
                            mybir.ActivationFunctionType.Identity,
                            bias=bt[:, m : m + 1],
                        )
                        nc.sync.dma_start(yt[ts(m, P), ts(n, 512)], ot[:])
                else:
                    for n in range(NT):
                        ps = psum.tile([P, 512], F32)
                        for k in range(KT):
                            nc.tensor.matmul(
                                ps[:],
                                lhsT=w[:, k, :],
                                rhs=xtiles[k][:, ts(n, 512)],
                                start=(k == 0),
                                stop=(k == KT - 1),
                            )
                        ot = opool.tile([P, 512], F32)
                        nc.scalar.activation(
                            ot[:], ps[:],
                            mybir.ActivationFunctionType.Identity,
                            bias=bt[:, m : m + 1],
                        )
                        nc.sync.dma_start(yt[ts(m, P), ts(n, 512)], ot[:])
    return nc


def prep_inputs(x, W, b, lora_a, lora_b, w_np=ml_dtypes.bfloat16,
                x_np=ml_dtypes.bfloat16):
    WT = W.T.astype(np.float32) + SCALE * (
        lora_b.astype(np.float32) @ lora_a.astype(np.float32)
    )
    wb = np.ascontiguousarray(
        WT.reshape(KT, P, MT, P).transpose(2, 1, 0, 3)
    ).astype(w_np)
    bias = np.ascontiguousarray(b.reshape(MT, P).T).astype(np.float32)
    in_maps = []
    for c in range(N_CORES):
        xt = np.ascontiguousarray(x[c].T).astype(x_np)
        in_maps.append({"xt": xt, "wb": wb, "bias": bias})
    return in_maps


def kernel(x, W, b, lora_a, lora_b):
    nc = build_nc()
    in_maps = prep_inputs(x, W, b, lora_a, lora_b)
    res = run_bass_kernel_spmd(nc, in_maps, core_ids=list(range(N_CORES)))
    out = np.empty((N_CORES, T, D), dtype=np.float32)
    for c in range(N_CORES):
        out[c] = res.results[c]["yt"].T
    return out
